# revision 1
# baseline (speedup 1.0000x reference)
"""Multi-head differential attention on 8 trn2 NeuronCores.

Sharding: core c handles batch b = c // 4 and heads [4g, 4g+4) where
g = c % 4 (batch x head-group parallel). All layout transposes happen
on the host; each core computes a partial [S, E] output (its heads'
contribution through the output projection), and the host sums the 4
partials per batch.

Device program (per core, SPMD):
  phase P (per head): qT1/qT2/kT1/kT2 = (W x^T) in [D, S] layout,
      V = x W^T in [S, D] layout, via PE matmuls streaming x^T chunks.
  phase A (per head): causal attention with scores kept transposed
      [k, q]: E = exp(s/sqrt(D)) (no max subtraction -- scores are
      O(5) so exp is safe in f32), row-of-ones matmul accumulates the
      softmax denominators, U = V^T E accumulates the unnormalized
      outputs, then diff = U1/l1 - lam*U2/l2 is formed with a
      partition-broadcast of the reciprocals.
  phase O: y_partial = oT^T @ woT via PE, streamed out to DRAM.
"""

import math
from contextlib import ExitStack

import numpy as np
import ml_dtypes

import concourse.bass as bass
import concourse.tile as tile
from concourse import bacc, mybir
from concourse import bass_utils

B, S, E = 2, 2048, 2048
H, D = 16, 128
HPC = 4                      # heads per core
N_CORES = 8
LAMBDA_INIT = 0.8 - 0.6 * math.exp(-0.3 * H)
SCALE = 1.0 / math.sqrt(D)

F32 = mybir.dt.float32
F32R = mybir.dt.float32r
BF16 = mybir.dt.bfloat16

NSB = S // 512               # proj s-blocks of 512
NE = E // 128                # contraction chunks of 128
NQB = S // 512               # attention q-blocks of 512
NST = S // 128               # s-tiles of 128

_cached = {}
DEBUG_TAPS = False
TRACE = False


def _build():
    nc = bacc.Bacc(
        "TRN2",
        target_bir_lowering=False,
        debug=False,
        enable_asserts=False,
        num_devices=N_CORES,
    )

    xT_d = nc.dram_tensor("xT", [E, S], F32R, kind="ExternalInput").ap()
    wT_d = nc.dram_tensor("wT", [4, E, HPC * D], F32R, kind="ExternalInput").ap()
    wvT_d = nc.dram_tensor("wvT", [E, HPC * D], F32R, kind="ExternalInput").ap()
    woT_d = nc.dram_tensor("woT", [HPC * D, E], F32R, kind="ExternalInput").ap()
    maskT_d = nc.dram_tensor("maskT", [128, 896], F32, kind="ExternalInput").ap()
    ones_d = nc.dram_tensor("ones", [128, 1], F32R, kind="ExternalInput").ap()
    ident_d = nc.dram_tensor("ident", [128, 128], F32R, kind="ExternalInput").ap()
    lam_d = nc.dram_tensor("lam", [1, 1], F32, kind="ExternalInput").ap()
    y_d = nc.dram_tensor("y", [S, E], F32, kind="ExternalOutput").ap()
    if DEBUG_TAPS:
        dbg_q_d = nc.dram_tensor("dbg_q", [128, 512], F32, kind="ExternalOutput").ap()
        dbg_k_d = nc.dram_tensor("dbg_k", [128, 512], F32, kind="ExternalOutput").ap()
        dbg_v_d = nc.dram_tensor("dbg_v", [128, 128], F32, kind="ExternalOutput").ap()
        dbg_e_d = nc.dram_tensor("dbg_e", [128, 512], F32, kind="ExternalOutput").ap()
        dbg_l_d = nc.dram_tensor("dbg_l", [1, 512], F32, kind="ExternalOutput").ap()
        dbg_u_d = nc.dram_tensor("dbg_u", [128, 512], F32, kind="ExternalOutput").ap()
        dbg_o_d = nc.dram_tensor("dbg_o", [128, 512], F32, kind="ExternalOutput").ap()

    with tile.TileContext(nc) as tc, ExitStack() as ctx:
        const = ctx.enter_context(tc.tile_pool(name="const", bufs=1))
        otp = ctx.enter_context(tc.tile_pool(name="otp", bufs=1))
        ps = ctx.enter_context(tc.tile_pool(name="ps", bufs=1, space="PSUM"))
        pactx = ExitStack()
        wp = pactx.enter_context(tc.tile_pool(name="wp", bufs=1))
        xp = pactx.enter_context(tc.tile_pool(name="xp", bufs=6))
        pout = pactx.enter_context(tc.tile_pool(name="pout", bufs=1))
        ep = pactx.enter_context(tc.tile_pool(name="ep", bufs=2))
        rp = pactx.enter_context(tc.tile_pool(name="rp", bufs=2))
        bp = pactx.enter_context(tc.tile_pool(name="bp", bufs=1))
        dp = pactx.enter_context(tc.tile_pool(name="dp", bufs=1))

        # constants
        maskT = const.tile([128, 896], F32)
        nc.sync.dma_start(out=maskT, in_=maskT_d)
        lam_sb = const.tile([1, 1], F32)
        nc.sync.dma_start(out=lam_sb, in_=lam_d)
        ones_col = const.tile([128, 1], F32R)
        nc.sync.dma_start(out=ones_col, in_=ones_d)
        ident = const.tile([128, 128], F32R)
        nc.sync.dma_start(out=ident, in_=ident_d)

        oT = []  # per head [128 d, S] f32
        for h in range(HPC):
            oT.append(otp.tile([128, S], F32R, tag=f"oT{h}", name=f"oT{h}"))

        for h in range(HPC):
            # ---- load this head's projection weights: [128 e, NE, 128 d]
            wjs = []
            for j in range(4):
                wj = wp.tile([128, NE, 128], F32R, tag=f"w{j}", name=f"wj{j}")
                nc.sync.dma_start(
                    out=wj,
                    in_=wT_d[j].rearrange("(c p) d -> p c d", p=128)[
                        :, :, h * 128:(h + 1) * 128
                    ],
                )
                wjs.append(wj)
            wv = wp.tile([128, NE, 128], F32R, tag="wv")
            nc.sync.dma_start(
                out=wv,
                in_=wvT_d.rearrange("(c p) d -> p c d", p=128)[
                    :, :, h * 128:(h + 1) * 128
                ],
            )

            # ---- projection outputs
            qT1 = pout.tile([128, S], F32R, tag="qT1")
            qT2 = pout.tile([128, S], F32R, tag="qT2")
            kT1 = pout.tile([128, S], F32R, tag="kT1")
            kT2 = pout.tile([128, S], F32R, tag="kT2")
            V = pout.tile([128, NST, 128], F32R, tag="V")

            # ---- projections, streaming xT chunks (one PSUM bank per
            # accumulation group: concurrent groups must not share a bank)
            vT = pout.tile([128, S], F32R, tag="vT")
            for sb in range(NSB):
                q1p = ps.tile([128, 512], F32, tag="t0")
                q2p = ps.tile([128, 512], F32, tag="t1")
                k1p = ps.tile([128, 512], F32, tag="t2")
                k2p = ps.tile([128, 512], F32, tag="t3")
                vTp = ps.tile([128, 512], F32, tag="t4")
                for e in range(NE):
                    xc = xp.tile([128, 512], F32R, tag="xc")
                    nc.sync.dma_start(
                        out=xc,
                        in_=xT_d[e * 128:(e + 1) * 128, sb * 512:(sb + 1) * 512],
                    )
                    st = e == 0
                    sp = e == NE - 1
                    nc.tensor.matmul(q1p, wjs[0][:, e, :], xc, start=st, stop=sp)
                    nc.tensor.matmul(q2p, wjs[1][:, e, :], xc, start=st, stop=sp)
                    nc.tensor.matmul(k1p, wjs[2][:, e, :], xc, start=st, stop=sp)
                    nc.tensor.matmul(k2p, wjs[3][:, e, :], xc, start=st, stop=sp)
                    nc.tensor.matmul(vTp, wv[:, e, :], xc, start=st, stop=sp)
                ssl = slice(sb * 512, (sb + 1) * 512)
                nc.scalar.copy(qT1[:, ssl], q1p)
                nc.scalar.copy(qT2[:, ssl], q2p)
                nc.scalar.copy(kT1[:, ssl], k1p)
                nc.scalar.copy(kT2[:, ssl], k2p)
                nc.scalar.copy(vT[:, ssl], vTp)
            # transpose vT [d, s] into natural V [s-tile, d] layout
            for stt in range(NST):
                tp = ps.tile([128, 128], F32R, tag="t5" if stt % 2 == 0 else "t6")
                nc.tensor.transpose(tp, vT[:, stt * 128:(stt + 1) * 128], ident)
                nc.vector.tensor_copy(V[:, stt, :], tp)

            if DEBUG_TAPS and h == 0:
                qt = dp.tile([128, 512], F32, tag="dbgQ")
                nc.vector.tensor_copy(qt, qT1[:, 0:512])
                nc.sync.dma_start(out=dbg_q_d, in_=qt)
                ktt = dp.tile([128, 512], F32, tag="dbgK")
                nc.vector.tensor_copy(ktt, kT1[:, 0:512])
                nc.sync.dma_start(out=dbg_k_d, in_=ktt)
                vt = dp.tile([128, 128], F32, tag="dbgV")
                nc.vector.tensor_copy(vt, V[:, 0, :])
                nc.sync.dma_start(out=dbg_v_d, in_=vt)
            # ---- attention
            for qb in range(NQB):
                qsl = slice(qb * 512, (qb + 1) * 512)
                U1 = ps.tile([128, 512], F32, tag="t2")
                U2 = ps.tile([128, 512], F32, tag="t3")
                l1 = ps.tile([1, 512], F32, tag="t4")
                l2 = ps.tile([1, 512], F32, tag="t5")
                nkt = 4 * qb + 4
                for kt in range(nkt):
                    s1 = ps.tile([128, 512], F32, tag="t0")
                    s2 = ps.tile([128, 512], F32, tag="t1")
                    ksl = slice(kt * 128, (kt + 1) * 128)
                    nc.tensor.matmul(s1, kT1[:, ksl], qT1[:, qsl])
                    nc.tensor.matmul(s2, kT2[:, ksl], qT2[:, qsl])
                    E1 = ep.tile([128, 512], F32R, tag="E1")
                    E2 = ep.tile([128, 512], F32R, tag="E2")
                    nc.scalar.activation(E1, s1,
                                         mybir.ActivationFunctionType.Exp,
                                         scale=SCALE)
                    nc.scalar.activation(E2, s2,
                                         mybir.ActivationFunctionType.Exp,
                                         scale=SCALE)
                    kl = kt - 4 * qb
                    if kl >= 0:
                        msl = slice(384 - kl * 128, 896 - kl * 128)
                        nc.vector.tensor_mul(E1, E1, maskT[:, msl])
                        nc.vector.tensor_mul(E2, E2, maskT[:, msl])
                    if DEBUG_TAPS and h == 0 and qb == 0 and kt == 0:
                        et = dp.tile([128, 512], F32, tag="dbgE")
                        nc.vector.tensor_copy(et, E1)
                        nc.sync.dma_start(out=dbg_e_d, in_=et)
                    st = kt == 0
                    sp = kt == nkt - 1
                    nc.tensor.matmul(U1, V[:, kt, :], E1,
                                     start=st, stop=sp, skip_group_check=True)
                    nc.tensor.matmul(l1, ones_col, E1,
                                     start=st, stop=sp, skip_group_check=True)
                    nc.tensor.matmul(U2, V[:, kt, :], E2,
                                     start=st, stop=sp, skip_group_check=True)
                    nc.tensor.matmul(l2, ones_col, E2,
                                     start=st, stop=sp, skip_group_check=True)
                if DEBUG_TAPS and h == 0 and qb == 0:
                    lt = rp.tile([1, 512], F32, tag="dbgl")
                    nc.vector.tensor_copy(lt, l1)
                    nc.sync.dma_start(out=dbg_l_d, in_=lt)
                    ut = dp.tile([128, 512], F32, tag="dbgU")
                    nc.vector.tensor_copy(ut, U1)
                    nc.sync.dma_start(out=dbg_u_d, in_=ut)
                # normalize + differential combine
                r1 = rp.tile([1, 512], F32, tag="r1")
                r2 = rp.tile([1, 512], F32, tag="r2")
                nc.vector.reciprocal(r1, l1)
                nc.vector.reciprocal(r2, l2)
                r2l = rp.tile([1, 512], F32, tag="r2l")
                nc.vector.tensor_scalar_mul(r2l, r2, lam_sb[0:1, 0:1])
                r1b = bp.tile([128, 512], F32, tag="r1b")
                r2b = bp.tile([128, 512], F32, tag="r2b")
                nc.gpsimd.partition_broadcast(r1b, r1)
                nc.gpsimd.partition_broadcast(r2b, r2l)
                d1 = dp.tile([128, 512], F32, tag="d1")
                d2 = dp.tile([128, 512], F32, tag="d2")
                nc.vector.tensor_mul(d1, U1, r1b)
                nc.vector.tensor_mul(d2, U2, r2b)
                nc.vector.tensor_sub(oT[h][:, qsl], d1, d2)
                if DEBUG_TAPS and h == 0 and qb == 0:
                    ot = dp.tile([128, 512], F32, tag="dbgO")
                    nc.vector.tensor_copy(ot, oT[h][:, qsl])
                    nc.sync.dma_start(out=dbg_o_d, in_=ot)

        # ---- output projection
        pactx.close()
        wop = ctx.enter_context(tc.tile_pool(name="wop", bufs=1))
        yp = ctx.enter_context(tc.tile_pool(name="yp", bufs=2))
        woT_sb = []
        for hh in range(HPC):
            t = wop.tile([128, E], F32R, tag=f"wo{hh}", name=f"woT{hh}")
            nc.sync.dma_start(out=t, in_=woT_d[hh * 128:(hh + 1) * 128, :])
            woT_sb.append(t)
        for stt in range(NST):
            ysb = yp.tile([128, E], F32, tag="ysb")
            ssl = slice(stt * 128, (stt + 1) * 128)
            for eb in range(4):
                ypp = ps.tile([128, 512], F32, tag="t6" if eb % 2 == 0 else "t7")
                for hh in range(HPC):
                    nc.tensor.matmul(
                        ypp,
                        oT[hh][:, ssl],
                        woT_sb[hh][:, eb * 512:(eb + 1) * 512],
                        start=(hh == 0), stop=(hh == HPC - 1),
                    )
                nc.scalar.copy(ysb[:, eb * 512:(eb + 1) * 512], ypp)
            nc.sync.dma_start(out=y_d[ssl, :], in_=ysb)

    nc.compile()
    return nc


def kernel(**inputs):
    x = np.asarray(inputs["x"], dtype=np.float32)
    wq = np.asarray(inputs["wq"], dtype=np.float32)
    wk = np.asarray(inputs["wk"], dtype=np.float32)
    wv = np.asarray(inputs["wv"], dtype=np.float32)
    wq2 = np.asarray(inputs["wq2"], dtype=np.float32)
    wk2 = np.asarray(inputs["wk2"], dtype=np.float32)
    wo = np.asarray(inputs["wo"], dtype=np.float32)
    lq1 = np.asarray(inputs["lambda_q1"], dtype=np.float64)
    lk1 = np.asarray(inputs["lambda_k1"], dtype=np.float64)
    lq2 = np.asarray(inputs["lambda_q2"], dtype=np.float64)
    lk2 = np.asarray(inputs["lambda_k2"], dtype=np.float64)

    lam = float(np.exp(np.sum(lq1 * lk1)) - np.exp(np.sum(lq2 * lk2)) + LAMBDA_INIT)

    if "nc" not in _cached:
        _cached["nc"] = _build()
    nc = _cached["nc"]

    maskT = (np.arange(896, dtype=np.int32)[None, :] - 384
             >= np.arange(128, dtype=np.int32)[:, None]).astype(np.float32)
    lam_arr = np.full((1, 1), lam, dtype=np.float32)

    xTs = [np.ascontiguousarray(x[b].T) for b in range(B)]
    in_maps = []
    for c in range(N_CORES):
        b = c // 4
        g = c % 4
        hs = slice(g * HPC * D, (g + 1) * HPC * D)
        wT = np.stack(
            [np.ascontiguousarray(w[hs, :].T) for w in (wq, wq2, wk, wk2)]
        )
        wvT = np.ascontiguousarray(wv[hs, :].T)
        woT = np.ascontiguousarray(wo[:, hs].T)
        in_maps.append({
            "xT": xTs[b],
            "wT": wT,
            "wvT": wvT,
            "woT": woT,
            "maskT": maskT,
            "ones": np.ones((128, 1), dtype=np.float32),
            "ident": np.eye(128, dtype=np.float32),
            "lam": lam_arr,
        })

    res = bass_utils.run_bass_kernel_spmd(nc, in_maps, core_ids=list(range(N_CORES)), trace=TRACE)
    _cached["last_result"] = res

    y = np.zeros((B, S, E), dtype=np.float32)
    for c in range(N_CORES):
        y[c // 4] += res.results[c]["y"]
    return y



# revision 13
# speedup vs baseline: 1.1707x; 1.1707x over previous
"""Multi-head differential attention on 8 trn2 NeuronCores.

Sharding: core c handles batch b = c // 4 and heads [4g, 4g+4) where
g = c % 4 (batch x head-group parallel). All layout transposes and
dtype conversion (f32 -> bf16) happen on the host; each core computes
a partial [S, E] output (its heads' contribution through the output
projection) in bf16, and the host sums the 4 partials per batch in
f32.

Device program (per core, SPMD), all matmul operands bf16 (enables
fast-weight-load so LDWEIGHTS overlaps the matmuls) with f32 PSUM
accumulation:

  V pass: V[s, d4] = x W_v^T computed directly in s-major layout
      (lhsT = xT e-chunk, rhs = wv e-chunk) -- no PE transposes.
  proj pass (per head, per proj q1/q2/k1/k2): qT/kT in [d, S] layout
      (lhsT = weight chunk, rhs = resident xT chunk).
  attention (per head, per 512-wide q block): scores kept transposed
      [k, q]; both streams' score tiles land in one 2-bank PSUM tile
      so a single ACT instruction computes exp of [128, 1024] into a
      bf16 E tile (no max subtraction -- scores are O(5) so exp is
      safe); causal mask applied only on block-diagonal tiles via one
      bf16 DVE multiply; U += V^T E and l += 1^T E accumulate in PSUM;
      normalization uses reciprocal_approx_fast + gpsimd partition
      broadcast, with the differential combine on DVE writing bf16 oT.
  out proj: y[s-tile, :] = sum_h oT_h^T wo_h, streamed out as bf16.
"""

import math
from contextlib import ExitStack

import numpy as np
import ml_dtypes

import concourse.bass as bass
import concourse.tile as tile
from concourse import bacc, mybir
from concourse import bass_utils

B, S, E = 2, 2048, 2048
H, D = 16, 128
HPC = 4                      # heads per core
N_CORES = 8
LAMBDA_INIT = 0.8 - 0.6 * math.exp(-0.3 * H)
SCALE = 1.0 / math.sqrt(D)

F32 = mybir.dt.float32
BF16 = mybir.dt.bfloat16

NE = E // 128                # contraction chunks of 128
NSB = S // 512               # 512-wide s blocks
NST = S // 128               # 128-wide s tiles
NQB = S // 512               # attention q blocks of 512

_cached = {}
TRACE = False


def _build():
    nc = bacc.Bacc(
        "TRN2",
        target_bir_lowering=False,
        debug=False,
        enable_asserts=False,
        num_devices=N_CORES,
    )

    x_d = nc.dram_tensor("x_lay", [128, NE, S], BF16, kind="ExternalInput").ap()
    wqk_d = nc.dram_tensor("wqk", [4, HPC, 128, NE, 128], BF16,
                           kind="ExternalInput").ap()
    wv_d = nc.dram_tensor("wv_lay", [128, NE, HPC * 128], BF16,
                          kind="ExternalInput").ap()
    wo_d = nc.dram_tensor("wo_lay", [HPC, 128, E], BF16,
                          kind="ExternalInput").ap()
    mask_d = nc.dram_tensor("maskcat", [128, 4, 1024], BF16,
                            kind="ExternalInput").ap()
    ones_d = nc.dram_tensor("ones", [128, 1], BF16, kind="ExternalInput").ap()
    lam_d = nc.dram_tensor("lam", [1, 1], F32, kind="ExternalInput").ap()
    y_d = nc.dram_tensor("y", [S, E], BF16, kind="ExternalOutput").ap()

    with tile.TileContext(nc) as tc, ExitStack() as ctx:
        const = ctx.enter_context(tc.tile_pool(name="const", bufs=1))
        qkp = ctx.enter_context(tc.tile_pool(name="qkp", bufs=1))
        vp = ctx.enter_context(tc.tile_pool(name="vp", bufs=1))
        otp = ctx.enter_context(tc.tile_pool(name="otp", bufs=1))

        # ---- constants
        ones_sb = const.tile([128, 1], BF16, name="ones")
        nc.sync.dma_start(out=ones_sb, in_=ones_d)
        lam_sb = const.tile([1, 1], F32, name="lam")
        nc.sync.dma_start(out=lam_sb, in_=lam_d)

        # ---- persistent SBUF tensors
        # qT1/qT2/kT1/kT2 per head, [128 d, S] bf16
        qk = [[qkp.tile([128, S], BF16, name=f"qk{p}h{h}", tag=f"qk{p}h{h}")
               for p in range(4)] for h in range(HPC)]
        # V in s-major layout: [128 s, st, 4h*128 d] bf16
        V = vp.tile([128, NST, HPC * 128], BF16, name="V")
        # per-head attention output, [128 d, S] bf16
        oT = [otp.tile([128, S], BF16, name=f"oT{h}", tag=f"oT{h}")
              for h in range(HPC)]

        # ---- phase-scoped pools (closed to free SBUF)
        pactx = ExitStack()
        xp = pactx.enter_context(tc.tile_pool(name="xp", bufs=1))
        wvp = pactx.enter_context(tc.tile_pool(name="wvp", bufs=1))
        wp = pactx.enter_context(tc.tile_pool(name="wp", bufs=2))
        pp = pactx.enter_context(tc.tile_pool(name="pp", bufs=1, space="PSUM"))

        # resident x, [128 e-lo, e-hi, S] bf16, split DMA over e-chunks
        xT = xp.tile([128, NE, S], BF16, name="xT")
        for e in range(NE):
            nc.sync.dma_start(out=xT[:, e, :], in_=x_d[:, e, :])

        # ---- V pass: V[st] = sum_e xT[e, st]^T @ wv[e]  ([128 s, 512 d4])
        wv_sb = wvp.tile([128, NE, HPC * 128], BF16, name="wv")
        nc.sync.dma_start(out=wv_sb, in_=wv_d)
        for st in range(NST):
            pv = pp.tile([128, 512], F32, tag="pa" if st % 2 == 0 else "pb")
            for e in range(NE):
                nc.tensor.matmul(pv, xT[:, e, st * 128:(st + 1) * 128],
                                 wv_sb[:, e, :], start=(e == 0), stop=(e == NE - 1))
            nc.vector.tensor_copy(V[:, st, :], pv)

        # ---- q/k projections: qk[h][p] = (W x^T) in [128 d, S]
        for h in range(HPC):
            for p in range(4):
                i = h * 4 + p
                w_sb = wp.tile([128, NE, 128], BF16, tag="w")
                nc.sync.dma_start(out=w_sb, in_=wqk_d[p, h])
                for sb in range(NSB):
                    acc = pp.tile([128, 512], F32,
                                  tag="pa" if (i * NSB + sb) % 2 == 0 else "pb")
                    for e in range(NE):
                        nc.tensor.matmul(acc, w_sb[:, e, :],
                                         xT[:, e, sb * 512:(sb + 1) * 512],
                                         start=(e == 0), stop=(e == NE - 1))
                    nc.vector.tensor_copy(qk[h][p][:, sb * 512:(sb + 1) * 512], acc)

        pactx.close()

        # ---- attention phase pools
        aactx = ExitStack()
        ps = aactx.enter_context(tc.tile_pool(name="ps", bufs=1, space="PSUM"))
        ep = aactx.enter_context(tc.tile_pool(name="ep", bufs=3))
        np_ = aactx.enter_context(tc.tile_pool(name="np", bufs=2))
        wop = aactx.enter_context(tc.tile_pool(name="wop", bufs=1))
        yp = aactx.enter_context(tc.tile_pool(name="yp", bufs=2))

        woT = [wop.tile([128, E], BF16, name=f"wo{h}", tag=f"wo{h}")
               for h in range(HPC)]
        for h in range(HPC):
            nc.sync.dma_start(out=woT[h], in_=wo_d[h])
        maskcat = wop.tile([128, 4, 1024], BF16, name="maskcat", tag="maskcat")
        nc.sync.dma_start(out=maskcat, in_=mask_d)

        for h in range(HPC):
            qT1, qT2, kT1, kT2 = qk[h]
            for qb in range(NQB):
                qsl = slice(qb * 512, (qb + 1) * 512)
                U1 = ps.tile([128, 512], F32, tag="u1")
                U2 = ps.tile([128, 512], F32, tag="u2")
                l1 = ps.tile([1, 512], F32, tag="l1")
                l2 = ps.tile([1, 512], F32, tag="l2")
                nkt = 4 * qb + 4
                for kt in range(nkt):
                    ksl = slice(kt * 128, (kt + 1) * 128)
                    scat = ps.tile([128, 1024], F32,
                                   tag="sca" if kt % 2 == 0 else "scb")
                    nc.tensor.matmul(scat[:, 0:512], kT1[:, ksl], qT1[:, qsl])
                    nc.tensor.matmul(scat[:, 512:1024], kT2[:, ksl], qT2[:, qsl])
                    Ecat = ep.tile([128, 1024], BF16, tag="E")
                    nc.scalar.activation(Ecat, scat,
                                         mybir.ActivationFunctionType.Exp,
                                         scale=SCALE)
                    kl = kt - 4 * qb
                    if kl >= 0:
                        nc.vector.tensor_mul(Ecat, Ecat, maskcat[:, kl, :])
                    st_ = (kt == 0)
                    sp_ = (kt == nkt - 1)
                    vsl = V[:, kt, h * 128:(h + 1) * 128]
                    nc.tensor.matmul(U1, vsl, Ecat[:, 0:512],
                                     start=st_, stop=sp_, skip_group_check=True)
                    nc.tensor.matmul(U2, vsl, Ecat[:, 512:1024],
                                     start=st_, stop=sp_, skip_group_check=True)
                    nc.tensor.matmul(l1, ones_sb, Ecat[:, 0:512],
                                     start=st_, stop=sp_, skip_group_check=True)
                    nc.tensor.matmul(l2, ones_sb, Ecat[:, 512:1024],
                                     start=st_, stop=sp_, skip_group_check=True)
                # normalization + differential combine
                r1 = np_.tile([1, 512], F32, tag="r1")
                r2 = np_.tile([1, 512], F32, tag="r2")
                nc.vector.reciprocal_approx_fast(out=r1, in_=l1)
                nc.vector.reciprocal_approx_fast(out=r2, in_=l2)
                r2l = np_.tile([1, 512], F32, tag="r2l")
                nc.vector.tensor_scalar_mul(r2l, r2, lam_sb[0:1, 0:1])
                r1b = np_.tile([128, 512], F32, tag="r1b")
                r2b = np_.tile([128, 512], F32, tag="r2b")
                nc.gpsimd.partition_broadcast(r1b, r1)
                nc.gpsimd.partition_broadcast(r2b, r2l)
                d1 = np_.tile([128, 512], BF16, tag="d1")
                d2 = np_.tile([128, 512], BF16, tag="d2")
                nc.vector.tensor_mul(d1, U1, r1b)
                nc.vector.tensor_mul(d2, U2, r2b)
                nc.vector.tensor_sub(oT[h][:, qsl], d1, d2)

        # ---- output projection: y[st] = sum_h oT_h[:, st]^T @ woT_h
        for st in range(NST):
            ysb = yp.tile([128, E], BF16, tag="ysb")
            ssl = slice(st * 128, (st + 1) * 128)
            for eb in range(4):
                ypp = ps.tile([128, 512], F32, tag="u1" if eb % 2 == 0 else "u2")
                for hh in range(HPC):
                    nc.tensor.matmul(ypp, oT[hh][:, ssl],
                                     woT[hh][:, eb * 512:(eb + 1) * 512],
                                     start=(hh == 0), stop=(hh == HPC - 1))
                nc.vector.tensor_copy(ysb[:, eb * 512:(eb + 1) * 512], ypp)
            nc.sync.dma_start(out=y_d[ssl, :], in_=ysb)

        aactx.close()

    nc.compile()
    return nc


def _lay_lhsT(w):
    # [E_contract, M] f32 -> [128 p, NE c, M] bf16 with e = c*128 + p
    e, m = w.shape
    return np.ascontiguousarray(
        w.reshape(NE, 128, m).transpose(1, 0, 2).astype(ml_dtypes.bfloat16))


def kernel(**inputs):
    x = np.asarray(inputs["x"], dtype=np.float32)
    wq = np.asarray(inputs["wq"], dtype=np.float32)
    wk = np.asarray(inputs["wk"], dtype=np.float32)
    wv = np.asarray(inputs["wv"], dtype=np.float32)
    wq2 = np.asarray(inputs["wq2"], dtype=np.float32)
    wk2 = np.asarray(inputs["wk2"], dtype=np.float32)
    wo = np.asarray(inputs["wo"], dtype=np.float32)
    lq1 = np.asarray(inputs["lambda_q1"], dtype=np.float64)
    lk1 = np.asarray(inputs["lambda_k1"], dtype=np.float64)
    lq2 = np.asarray(inputs["lambda_q2"], dtype=np.float64)
    lk2 = np.asarray(inputs["lambda_k2"], dtype=np.float64)

    lam = float(np.exp(np.sum(lq1 * lk1)) - np.exp(np.sum(lq2 * lk2)) + LAMBDA_INIT)

    if "nc" not in _cached:
        _cached["nc"] = _build()
    nc = _cached["nc"]

    # block-diagonal causal mask tiles: for kl in 0..3, M_kl[i, j] = j >= kl*128+i,
    # duplicated across the two attention streams -> [128, 4*1024]
    j = np.arange(512, dtype=np.int32)[None, :]
    i = np.arange(128, dtype=np.int32)[:, None]
    mk = np.stack(
        [np.tile((j >= kl * 128 + i), (1, 2)) for kl in range(4)], axis=1)
    maskcat = np.ascontiguousarray(mk.astype(ml_dtypes.bfloat16))
    lam_arr = np.full((1, 1), lam, dtype=np.float32)
    ones = np.ones((128, 1), dtype=ml_dtypes.bfloat16)

    # per-batch x layout: [128 p, NE c, S] with e = c*128 + p
    x_lays = [_lay_lhsT(x[b].T) for b in range(B)]

    in_maps = []
    for c in range(N_CORES):
        b = c // 4
        g = c % 4
        hs = g * HPC * D
        # wqk[p, h] = lhsT chunk layout of W[o_slice, :]^T ([E, 128] -> [128, NE, 128])
        wqk = np.stack([
            np.stack([_lay_lhsT(w[hs + h * D:hs + (h + 1) * D, :].T)
                      for h in range(HPC)])
            for w in (wq, wq2, wk, wk2)])
        wv_lay = _lay_lhsT(wv[hs:hs + HPC * D, :].T)          # [128, NE, 512]
        wo_lay = np.stack([
            np.ascontiguousarray(
                wo[:, hs + h * D:hs + (h + 1) * D].T.astype(ml_dtypes.bfloat16))
            for h in range(HPC)])                              # [4, 128, E]
        in_maps.append({
            "x_lay": x_lays[b],
            "wqk": wqk,
            "wv_lay": wv_lay,
            "wo_lay": wo_lay,
            "maskcat": maskcat,
            "ones": ones,
            "lam": lam_arr,
        })

    res = bass_utils.run_bass_kernel_spmd(
        nc, in_maps, core_ids=list(range(N_CORES)), trace=TRACE)
    _cached["last_result"] = res

    y = np.zeros((B, S, E), dtype=np.float32)
    for c in range(N_CORES):
        y[c // 4] += np.asarray(res.results[c]["y"], dtype=np.float32)
    return y


# revision 20
# speedup vs baseline: 1.2280x; 1.0489x over previous
"""Multi-head differential attention on 8 trn2 NeuronCores.

Sharding: core c handles batch b = c // 4 and heads [4g, 4g+4) where
g = c % 4 (batch x head-group parallel). All layout transposes and
dtype conversion (f32 -> bf16) happen on the host; each core computes
a partial [S, E] output (its heads' contribution through the output
projection) in bf16, and the host sums the 4 partials per batch in
f32.

Device program (per core, SPMD), all matmul operands bf16 (enables
fast-weight-load so LDWEIGHTS overlaps the matmuls) with f32 PSUM
accumulation:

  V pass: V[s, d4] = x W_v^T computed directly in s-major layout
      (lhsT = xT e-chunk, rhs = wv e-chunk) -- no PE transposes.
  proj pass (per head, per proj q1/q2/k1/k2): qT/kT in [d, S] layout
      (lhsT = weight chunk, rhs = resident xT chunk).
  attention (per head, per 512-wide q block): scores kept transposed
      [k, q]; both streams' score tiles land in one 2-bank PSUM tile
      so a single ACT instruction computes exp of [128, 1024] into a
      bf16 E tile (no max subtraction -- scores are O(5) so exp is
      safe); causal mask applied only on block-diagonal tiles via one
      bf16 DVE multiply; U += V^T E and l += 1^T E accumulate in PSUM;
      normalization uses reciprocal_approx_fast + gpsimd partition
      broadcast, with the differential combine on DVE writing bf16 oT.
  out proj: y[s-tile, :] = sum_h oT_h^T wo_h, streamed out as bf16.
"""

import math
from contextlib import ExitStack

import numpy as np
import ml_dtypes

import concourse.bass as bass
import concourse.tile as tile
from concourse import bacc, mybir
from concourse import bass_utils

B, S, E = 2, 2048, 2048
H, D = 16, 128
HPC = 4                      # heads per core
N_CORES = 8
LAMBDA_INIT = 0.8 - 0.6 * math.exp(-0.3 * H)
SCALE = 1.0 / math.sqrt(D)

F32 = mybir.dt.float32
BF16 = mybir.dt.bfloat16

NE = E // 128                # contraction chunks of 128
NSB = S // 512               # 512-wide s blocks
NST = S // 128               # 128-wide s tiles
NQB = S // 512               # attention q blocks of 512

_cached = {}
TRACE = False


def _build():
    nc = bacc.Bacc(
        "TRN2",
        target_bir_lowering=False,
        debug=False,
        enable_asserts=False,
        num_devices=N_CORES,
    )

    x_d = nc.dram_tensor("x_lay", [128, NE, S], BF16, kind="ExternalInput").ap()
    wqk_d = nc.dram_tensor("wqk", [4, HPC, 128, NE, 128], BF16,
                           kind="ExternalInput").ap()
    wv_d = nc.dram_tensor("wv_lay", [128, NE, HPC * 128], BF16,
                          kind="ExternalInput").ap()
    wo_d = nc.dram_tensor("wo_lay", [HPC, 128, E], BF16,
                          kind="ExternalInput").ap()
    mask_d = nc.dram_tensor("mask1", [128, 128], BF16,
                            kind="ExternalInput").ap()
    ones_d = nc.dram_tensor("ones", [128, 1], BF16, kind="ExternalInput").ap()
    lam_d = nc.dram_tensor("lam", [1, 1], F32, kind="ExternalInput").ap()
    y_d = nc.dram_tensor("y", [S, E], BF16, kind="ExternalOutput").ap()

    with tile.TileContext(nc) as tc, ExitStack() as ctx:
        const = ctx.enter_context(tc.tile_pool(name="const", bufs=1))
        qkp = ctx.enter_context(tc.tile_pool(name="qkp", bufs=1))
        vp = ctx.enter_context(tc.tile_pool(name="vp", bufs=1))
        otp = ctx.enter_context(tc.tile_pool(name="otp", bufs=1))

        # ---- constants
        ones_sb = const.tile([128, 1], BF16, name="ones")
        nc.sync.dma_start(out=ones_sb, in_=ones_d)
        lam_sb = const.tile([1, 1], F32, name="lam")
        nc.sync.dma_start(out=lam_sb, in_=lam_d)

        # ---- persistent SBUF tensors
        # qT1/qT2/kT1/kT2 per head, [128 d, S] bf16
        qk = [[qkp.tile([128, S], BF16, name=f"qk{p}h{h}", tag=f"qk{p}h{h}")
               for p in range(4)] for h in range(HPC)]
        # V in s-major layout: [128 s, st, 4h*128 d] bf16
        V = vp.tile([128, NST, HPC * 128], BF16, name="V")
        # per-head attention output, [128 d, S] bf16
        oT = [otp.tile([128, S], BF16, name=f"oT{h}", tag=f"oT{h}")
              for h in range(HPC)]

        # ---- phase-scoped pools (closed to free SBUF)
        pactx = ExitStack()
        xp = pactx.enter_context(tc.tile_pool(name="xp", bufs=1))
        wvp = pactx.enter_context(tc.tile_pool(name="wvp", bufs=1))
        wp = pactx.enter_context(tc.tile_pool(name="wp", bufs=2))
        pp = pactx.enter_context(tc.tile_pool(name="pp", bufs=1, space="PSUM"))

        # resident x, [128 e-lo, e-hi, S] bf16; DMA in s-block-major order so
        # the first projection groups can start before the full load lands
        xT = xp.tile([128, NE, S], BF16, name="xT")
        for sb in range(NSB):
            for eq in range(4):
                nc.sync.dma_start(
                    out=xT[:, eq * 4:(eq + 1) * 4, sb * 512:(sb + 1) * 512],
                    in_=x_d[:, eq * 4:(eq + 1) * 4, sb * 512:(sb + 1) * 512])
        wv_sb = wvp.tile([128, NE, HPC * 128], BF16, name="wv")
        nc.sync.dma_start(out=wv_sb, in_=wv_d)

        pcount = 0

        def _qk_proj(h):
            nonlocal pcount
            for p in range(4):
                w_sb = wp.tile([128, NE, 128], BF16, tag="w")
                nc.sync.dma_start(out=w_sb, in_=wqk_d[p, h])
                for sb in range(NSB):
                    acc = pp.tile([128, 512], F32,
                                  tag="pa" if pcount % 2 == 0 else "pb")
                    pcount += 1
                    for e in range(NE):
                        nc.tensor.matmul(acc, w_sb[:, e, :],
                                         xT[:, e, sb * 512:(sb + 1) * 512],
                                         start=(e == 0), stop=(e == NE - 1))
                    nc.vector.tensor_copy(qk[h][p][:, sb * 512:(sb + 1) * 512], acc)

        # head 0 projections first (need only one s-block of x to start),
        # then the V pass (needs all of x), then the remaining heads
        _qk_proj(0)
        for st in range(NST):
            pv = pp.tile([128, 512], F32, tag="pa" if pcount % 2 == 0 else "pb")
            pcount += 1
            for e in range(NE):
                nc.tensor.matmul(pv, xT[:, e, st * 128:(st + 1) * 128],
                                 wv_sb[:, e, :], start=(e == 0), stop=(e == NE - 1))
            nc.vector.tensor_copy(V[:, st, :], pv)
        for h in range(1, HPC):
            _qk_proj(h)

        pactx.close()

        # ---- attention phase pools
        aactx = ExitStack()
        ps = aactx.enter_context(tc.tile_pool(name="ps", bufs=1, space="PSUM"))
        ep = aactx.enter_context(tc.tile_pool(name="ep", bufs=4))
        np_ = aactx.enter_context(tc.tile_pool(name="np", bufs=2))
        wop = aactx.enter_context(tc.tile_pool(name="wop", bufs=1))
        yp = aactx.enter_context(tc.tile_pool(name="yp", bufs=2))

        woT = [wop.tile([128, E], BF16, name=f"wo{h}", tag=f"wo{h}")
               for h in range(HPC)]
        for h in range(HPC):
            nc.sync.dma_start(out=woT[h], in_=wo_d[h])
        mask1 = wop.tile([128, 128], BF16, name="mask1", tag="mask1")
        nc.sync.dma_start(out=mask1, in_=mask_d)

        for h in range(HPC):
            qT1, qT2, kT1, kT2 = qk[h]
            for qb in range(NQB):
                qsl = slice(qb * 512, (qb + 1) * 512)
                U1 = ps.tile([128, 512], F32, tag="u1")
                U2 = ps.tile([128, 512], F32, tag="u2")
                l1 = ps.tile([1, 512], F32, tag="l1")
                l2 = ps.tile([1, 512], F32, tag="l2")
                nkt = 4 * qb + 4
                for kt in range(nkt):
                    ksl = slice(kt * 128, (kt + 1) * 128)
                    kl = kt - 4 * qb
                    # on block-diagonal tiles, skip the fully-masked
                    # q columns: only q >= kl*128 contributes
                    qo = max(kl, 0) * 128
                    n = 512 - qo
                    qsl2 = slice(qb * 512 + qo, (qb + 1) * 512)
                    scat = ps.tile([128, 1024], F32,
                                   tag="sca" if kt % 2 == 0 else "scb")
                    nc.tensor.matmul(scat[:, 0:n], kT1[:, ksl], qT1[:, qsl2])
                    nc.tensor.matmul(scat[:, 512:512 + n], kT2[:, ksl],
                                     qT2[:, qsl2])
                    Ecat = ep.tile([128, 1024], BF16, tag="E")
                    # single exp over both streams' banks; the unwritten
                    # middle region is junk that no consumer reads
                    nc.scalar.activation(Ecat, scat,
                                         mybir.ActivationFunctionType.Exp,
                                         scale=SCALE)
                    if kl >= 0:
                        # mask the leading 128 q columns (the k-tile's own
                        # diagonal 128x128 block) of each stream
                        nc.vector.tensor_mul(Ecat[:, 0:128], Ecat[:, 0:128],
                                             mask1)
                        nc.vector.tensor_mul(Ecat[:, 512:640], Ecat[:, 512:640],
                                             mask1)
                    st_ = (kt == 0)
                    sp_ = (kt == nkt - 1)
                    vsl = V[:, kt, h * 128:(h + 1) * 128]
                    nc.tensor.matmul(U1[:, qo:512], vsl, Ecat[:, 0:n],
                                     start=st_, stop=sp_, skip_group_check=True)
                    nc.tensor.matmul(U2[:, qo:512], vsl, Ecat[:, 512:512 + n],
                                     start=st_, stop=sp_, skip_group_check=True)
                    nc.tensor.matmul(l1[:, qo:512], ones_sb, Ecat[:, 0:n],
                                     start=st_, stop=sp_, skip_group_check=True)
                    nc.tensor.matmul(l2[:, qo:512], ones_sb, Ecat[:, 512:512 + n],
                                     start=st_, stop=sp_, skip_group_check=True)
                # copy U out of PSUM promptly to release the banks, then
                # normalize + differential combine off the critical path
                u1c = np_.tile([128, 512], F32, tag="u1c")
                u2c = np_.tile([128, 512], F32, tag="u2c")
                nc.vector.tensor_copy(u1c, U1)
                nc.vector.tensor_copy(u2c, U2)
                r1 = np_.tile([1, 512], F32, tag="r1")
                r2 = np_.tile([1, 512], F32, tag="r2")
                nc.vector.reciprocal_approx_fast(out=r1, in_=l1)
                nc.vector.reciprocal_approx_fast(out=r2, in_=l2)
                r2l = np_.tile([1, 512], F32, tag="r2l")
                nc.vector.tensor_scalar_mul(r2l, r2, lam_sb[0:1, 0:1])
                r1b = np_.tile([128, 512], F32, tag="r1b")
                r2b = np_.tile([128, 512], F32, tag="r2b")
                nc.gpsimd.partition_broadcast(r1b, r1)
                nc.gpsimd.partition_broadcast(r2b, r2l)
                d1 = np_.tile([128, 512], BF16, tag="d1")
                d2 = np_.tile([128, 512], BF16, tag="d2")
                nc.vector.tensor_mul(d1, u1c, r1b)
                nc.vector.tensor_mul(d2, u2c, r2b)
                nc.vector.tensor_sub(oT[h][:, qsl], d1, d2)

        # ---- output projection: y[st] = sum_h oT_h[:, st]^T @ woT_h
        for st in range(NST):
            ysb = yp.tile([128, E], BF16, tag="ysb")
            ssl = slice(st * 128, (st + 1) * 128)
            for eb in range(4):
                ypp = ps.tile([128, 512], F32, tag="u1" if eb % 2 == 0 else "u2")
                for hh in range(HPC):
                    nc.tensor.matmul(ypp, oT[hh][:, ssl],
                                     woT[hh][:, eb * 512:(eb + 1) * 512],
                                     start=(hh == 0), stop=(hh == HPC - 1))
                nc.vector.tensor_copy(ysb[:, eb * 512:(eb + 1) * 512], ypp)
            nc.sync.dma_start(out=y_d[ssl, :], in_=ysb)

        aactx.close()

    nc.compile()
    return nc


def _lay_lhsT(w):
    # [E_contract, M] f32 -> [128 p, NE c, M] bf16 with e = c*128 + p
    e, m = w.shape
    return np.ascontiguousarray(
        w.reshape(NE, 128, m).transpose(1, 0, 2).astype(ml_dtypes.bfloat16))


def kernel(**inputs):
    x = np.asarray(inputs["x"], dtype=np.float32)
    wq = np.asarray(inputs["wq"], dtype=np.float32)
    wk = np.asarray(inputs["wk"], dtype=np.float32)
    wv = np.asarray(inputs["wv"], dtype=np.float32)
    wq2 = np.asarray(inputs["wq2"], dtype=np.float32)
    wk2 = np.asarray(inputs["wk2"], dtype=np.float32)
    wo = np.asarray(inputs["wo"], dtype=np.float32)
    lq1 = np.asarray(inputs["lambda_q1"], dtype=np.float64)
    lk1 = np.asarray(inputs["lambda_k1"], dtype=np.float64)
    lq2 = np.asarray(inputs["lambda_q2"], dtype=np.float64)
    lk2 = np.asarray(inputs["lambda_k2"], dtype=np.float64)

    lam = float(np.exp(np.sum(lq1 * lk1)) - np.exp(np.sum(lq2 * lk2)) + LAMBDA_INIT)

    if "nc" not in _cached:
        _cached["nc"] = _build()
    nc = _cached["nc"]

    # 128x128 lower-triangular mask for the diagonal blocks
    j = np.arange(128, dtype=np.int32)[None, :]
    i = np.arange(128, dtype=np.int32)[:, None]
    mask1 = np.ascontiguousarray((j >= i).astype(ml_dtypes.bfloat16))
    lam_arr = np.full((1, 1), lam, dtype=np.float32)
    ones = np.ones((128, 1), dtype=ml_dtypes.bfloat16)

    # per-batch x layout: [128 p, NE c, S] with e = c*128 + p
    x_lays = [_lay_lhsT(x[b].T) for b in range(B)]

    in_maps = []
    for c in range(N_CORES):
        b = c // 4
        g = c % 4
        hs = g * HPC * D
        # wqk[p, h] = lhsT chunk layout of W[o_slice, :]^T ([E, 128] -> [128, NE, 128])
        wqk = np.stack([
            np.stack([_lay_lhsT(w[hs + h * D:hs + (h + 1) * D, :].T)
                      for h in range(HPC)])
            for w in (wq, wq2, wk, wk2)])
        wv_lay = _lay_lhsT(wv[hs:hs + HPC * D, :].T)          # [128, NE, 512]
        wo_lay = np.stack([
            np.ascontiguousarray(
                wo[:, hs + h * D:hs + (h + 1) * D].T.astype(ml_dtypes.bfloat16))
            for h in range(HPC)])                              # [4, 128, E]
        in_maps.append({
            "x_lay": x_lays[b],
            "wqk": wqk,
            "wv_lay": wv_lay,
            "wo_lay": wo_lay,
            "mask1": mask1,
            "ones": ones,
            "lam": lam_arr,
        })

    res = bass_utils.run_bass_kernel_spmd(
        nc, in_maps, core_ids=list(range(N_CORES)), trace=TRACE)
    _cached["last_result"] = res

    y = np.zeros((B, S, E), dtype=np.float32)
    for c in range(N_CORES):
        y[c // 4] += np.asarray(res.results[c]["y"], dtype=np.float32)
    return y


# revision 23
# speedup vs baseline: 1.3143x; 1.0703x over previous
"""Multi-head differential attention on 8 trn2 NeuronCores.

Sharding: core c handles batch b = c // 4 and heads [4g, 4g+4) where
g = c % 4 (batch x head-group parallel). All layout transposes and
dtype conversion (f32 -> bf16) happen on the host; each core computes
a partial [S, E] output (its heads' contribution through the output
projection) in bf16, and the host sums the 4 partials per batch in
f32.

Device program (per core, SPMD), all matmul operands bf16 (enables
fast-weight-load so LDWEIGHTS overlaps the matmuls) with f32 PSUM
accumulation:

  V pass: V[s, d4] = x W_v^T computed directly in s-major layout
      (lhsT = xT e-chunk, rhs = wv e-chunk) -- no PE transposes.
  proj pass (per head, per proj q1/q2/k1/k2): qT/kT in [d, S] layout
      (lhsT = weight chunk, rhs = resident xT chunk).
  attention (per head, per 512-wide q block): scores kept transposed
      [k, q]; both streams' score tiles land in one 2-bank PSUM tile
      so a single ACT instruction computes exp of [128, 1024] into a
      bf16 E tile (no max subtraction -- scores are O(5) so exp is
      safe); causal mask applied only on block-diagonal tiles via one
      bf16 DVE multiply; U += V^T E and l += 1^T E accumulate in PSUM;
      normalization uses reciprocal_approx_fast + gpsimd partition
      broadcast, with the differential combine on DVE writing bf16 oT.
  out proj: y[s-tile, :] = sum_h oT_h^T wo_h, streamed out as bf16.
"""

import math
from contextlib import ExitStack

import numpy as np
import ml_dtypes

import concourse.bass as bass
import concourse.tile as tile
from concourse import bacc, mybir
from concourse import bass_utils

B, S, E = 2, 2048, 2048
H, D = 16, 128
HPC = 4                      # heads per core
N_CORES = 8
LAMBDA_INIT = 0.8 - 0.6 * math.exp(-0.3 * H)
SCALE = 1.0 / math.sqrt(D)

F32 = mybir.dt.float32
BF16 = mybir.dt.bfloat16

NE = E // 128                # contraction chunks of 128
NSB = S // 512               # 512-wide s blocks
NST = S // 128               # 128-wide s tiles
NQB = S // 512               # attention q blocks of 512

_cached = {}
TRACE = False


def _build():
    nc = bacc.Bacc(
        "TRN2",
        target_bir_lowering=False,
        debug=False,
        enable_asserts=False,
        num_devices=N_CORES,
    )

    x_d = nc.dram_tensor("x_lay", [128, NE, S], BF16, kind="ExternalInput").ap()
    wqk_d = nc.dram_tensor("wqk", [4, HPC, 128, NE, 128], BF16,
                           kind="ExternalInput").ap()
    wv_d = nc.dram_tensor("wv_lay", [128, NE, HPC * 128], BF16,
                          kind="ExternalInput").ap()
    wo_d = nc.dram_tensor("wo_lay", [HPC, 128, E], BF16,
                          kind="ExternalInput").ap()
    mask_d = nc.dram_tensor("mask1", [128, 128], BF16,
                            kind="ExternalInput").ap()
    ones_d = nc.dram_tensor("ones", [128, 1], BF16, kind="ExternalInput").ap()
    lam_d = nc.dram_tensor("lam", [1, 1], F32, kind="ExternalInput").ap()
    y_d = nc.dram_tensor("y", [S, E], BF16, kind="ExternalOutput").ap()

    with tile.TileContext(nc) as tc, ExitStack() as ctx:
        const = ctx.enter_context(tc.tile_pool(name="const", bufs=1))
        qkp = ctx.enter_context(tc.tile_pool(name="qkp", bufs=1))
        vp = ctx.enter_context(tc.tile_pool(name="vp", bufs=1))
        otp = ctx.enter_context(tc.tile_pool(name="otp", bufs=1))

        # ---- constants
        ones_sb = const.tile([128, 1], BF16, name="ones")
        nc.sync.dma_start(out=ones_sb, in_=ones_d)
        lam_sb = const.tile([1, 1], F32, name="lam")
        nc.sync.dma_start(out=lam_sb, in_=lam_d)

        # ---- persistent SBUF tensors
        # qT1/qT2/kT1/kT2 per head, [128 d, S] bf16
        qk = [[qkp.tile([128, S], BF16, name=f"qk{p}h{h}", tag=f"qk{p}h{h}")
               for p in range(4)] for h in range(HPC)]
        # V in s-major layout: [128 s, st, 4h*128 d] bf16
        V = vp.tile([128, NST, HPC * 128], BF16, name="V")
        # per-head attention output, [128 d, S] bf16
        oT = [otp.tile([128, S], BF16, name=f"oT{h}", tag=f"oT{h}")
              for h in range(HPC)]

        # ---- phase-scoped pools (closed to free SBUF)
        pactx = ExitStack()
        xp = pactx.enter_context(tc.tile_pool(name="xp", bufs=1))
        wvp = pactx.enter_context(tc.tile_pool(name="wvp", bufs=1))
        wp = pactx.enter_context(tc.tile_pool(name="wp", bufs=2))
        pp = pactx.enter_context(tc.tile_pool(name="pp", bufs=1, space="PSUM"))

        # resident x, [128 e-lo, e-hi, S] bf16; the first projection group
        # needs w(h0,p0) and x s-block 0, so issue those first, split small
        # so they spread across DMA queues, then stream the rest s-block-major
        xT = xp.tile([128, NE, S], BF16, name="xT")
        w00 = wp.tile([128, NE, 128], BF16, tag="w")
        for eq in range(4):
            esl = slice(eq * 4, (eq + 1) * 4)
            nc.sync.dma_start(out=w00[:, esl, :], in_=wqk_d[0, 0, :, esl, :])
            nc.sync.dma_start(out=xT[:, esl, 0:512], in_=x_d[:, esl, 0:512])
        for sb in range(1, NSB):
            for eq in range(4):
                nc.sync.dma_start(
                    out=xT[:, eq * 4:(eq + 1) * 4, sb * 512:(sb + 1) * 512],
                    in_=x_d[:, eq * 4:(eq + 1) * 4, sb * 512:(sb + 1) * 512])
        wv_sb = wvp.tile([128, NE, HPC * 128], BF16, name="wv")
        nc.sync.dma_start(out=wv_sb, in_=wv_d)

        pcount = 0

        def _qk_proj(h):
            nonlocal pcount
            for p in range(4):
                if h == 0 and p == 0:
                    w_sb = w00
                else:
                    w_sb = wp.tile([128, NE, 128], BF16, tag="w")
                    nc.sync.dma_start(out=w_sb, in_=wqk_d[p, h])
                for sb in range(NSB):
                    acc = pp.tile([128, 512], F32,
                                  tag="pa" if pcount % 2 == 0 else "pb")
                    pcount += 1
                    for e in range(NE):
                        nc.tensor.matmul(acc, w_sb[:, e, :],
                                         xT[:, e, sb * 512:(sb + 1) * 512],
                                         start=(e == 0), stop=(e == NE - 1))
                    nc.vector.tensor_copy(qk[h][p][:, sb * 512:(sb + 1) * 512], acc)

        # head 0 projections first (need only one s-block of x to start),
        # then the V pass (needs all of x), then the remaining heads
        _qk_proj(0)
        for st in range(NST):
            pv = pp.tile([128, 512], F32, tag="pa" if pcount % 2 == 0 else "pb")
            pcount += 1
            for e in range(NE):
                nc.tensor.matmul(pv, xT[:, e, st * 128:(st + 1) * 128],
                                 wv_sb[:, e, :], start=(e == 0), stop=(e == NE - 1))
            nc.vector.tensor_copy(V[:, st, :], pv)
        for h in range(1, HPC):
            _qk_proj(h)

        pactx.close()

        # ---- attention phase pools
        aactx = ExitStack()
        ps = aactx.enter_context(tc.tile_pool(name="ps", bufs=1, space="PSUM"))
        ep = aactx.enter_context(tc.tile_pool(name="ep", bufs=4))
        np_ = aactx.enter_context(tc.tile_pool(name="np", bufs=2))
        wop = aactx.enter_context(tc.tile_pool(name="wop", bufs=1))
        yp = aactx.enter_context(tc.tile_pool(name="yp", bufs=2))

        woT = [wop.tile([128, E], BF16, name=f"wo{h}", tag=f"wo{h}")
               for h in range(HPC)]
        for h in range(HPC):
            nc.sync.dma_start(out=woT[h], in_=wo_d[h])
        mask1 = wop.tile([128, 128], BF16, name="mask1", tag="mask1")
        nc.sync.dma_start(out=mask1, in_=mask_d)

        for h in range(HPC):
            qT1, qT2, kT1, kT2 = qk[h]
            for qb in range(NQB):
                qsl = slice(qb * 512, (qb + 1) * 512)
                U1 = ps.tile([128, 512], F32, tag="u1")
                U2 = ps.tile([128, 512], F32, tag="u2")
                l1 = ps.tile([1, 512], F32, tag="l1")
                l2 = ps.tile([1, 512], F32, tag="l2")
                nkt = 4 * qb + 4
                ecats = {}

                def _consume(kt):
                    # U/l accumulation for tile kt (runs one kt behind the
                    # score/exp stage so the exp latency is fully hidden)
                    qo, n, Ecat = ecats.pop(kt)
                    st_ = (kt == 0)
                    sp_ = (kt == nkt - 1)
                    vsl = V[:, kt, h * 128:(h + 1) * 128]
                    nc.tensor.matmul(U1[:, qo:512], vsl, Ecat[:, 0:n],
                                     start=st_, stop=sp_, skip_group_check=True)
                    nc.tensor.matmul(U2[:, qo:512], vsl, Ecat[:, 512:512 + n],
                                     start=st_, stop=sp_, skip_group_check=True)
                    nc.tensor.matmul(l1[:, qo:512], ones_sb, Ecat[:, 0:n],
                                     start=st_, stop=sp_, skip_group_check=True)
                    nc.tensor.matmul(l2[:, qo:512], ones_sb, Ecat[:, 512:512 + n],
                                     start=st_, stop=sp_, skip_group_check=True)

                for kt in range(nkt):
                    ksl = slice(kt * 128, (kt + 1) * 128)
                    kl = kt - 4 * qb
                    # on block-diagonal tiles, skip the fully-masked
                    # q columns: only q >= kl*128 contributes
                    qo = max(kl, 0) * 128
                    n = 512 - qo
                    qsl2 = slice(qb * 512 + qo, (qb + 1) * 512)
                    scat = ps.tile([128, 1024], F32,
                                   tag="sca" if kt % 2 == 0 else "scb")
                    nc.tensor.matmul(scat[:, 0:n], kT1[:, ksl], qT1[:, qsl2])
                    nc.tensor.matmul(scat[:, 512:512 + n], kT2[:, ksl],
                                     qT2[:, qsl2])
                    Ecat = ep.tile([128, 1024], BF16, tag="E")
                    # single exp over both streams' banks; the unwritten
                    # middle region is junk that no consumer reads
                    nc.scalar.activation(Ecat, scat,
                                         mybir.ActivationFunctionType.Exp,
                                         scale=SCALE)
                    if kl >= 0:
                        # mask the leading 128 q columns (the k-tile's own
                        # diagonal 128x128 block) of each stream
                        nc.vector.tensor_mul(Ecat[:, 0:128], Ecat[:, 0:128],
                                             mask1)
                        nc.vector.tensor_mul(Ecat[:, 512:640], Ecat[:, 512:640],
                                             mask1)
                    ecats[kt] = (qo, n, Ecat)
                    if kt >= 1:
                        _consume(kt - 1)
                _consume(nkt - 1)
                # copy U out of PSUM promptly to release the banks, then
                # normalize + differential combine off the critical path
                u1c = np_.tile([128, 512], F32, tag="u1c")
                u2c = np_.tile([128, 512], F32, tag="u2c")
                nc.vector.tensor_copy(u1c, U1)
                nc.vector.tensor_copy(u2c, U2)
                r1 = np_.tile([1, 512], F32, tag="r1")
                r2 = np_.tile([1, 512], F32, tag="r2")
                nc.vector.reciprocal_approx_fast(out=r1, in_=l1)
                nc.vector.reciprocal_approx_fast(out=r2, in_=l2)
                r2l = np_.tile([1, 512], F32, tag="r2l")
                nc.vector.tensor_scalar_mul(r2l, r2, lam_sb[0:1, 0:1])
                r1b = np_.tile([128, 512], F32, tag="r1b")
                r2b = np_.tile([128, 512], F32, tag="r2b")
                nc.gpsimd.partition_broadcast(r1b, r1)
                nc.gpsimd.partition_broadcast(r2b, r2l)
                d1 = np_.tile([128, 512], BF16, tag="d1")
                d2 = np_.tile([128, 512], BF16, tag="d2")
                nc.vector.tensor_mul(d1, u1c, r1b)
                nc.vector.tensor_mul(d2, u2c, r2b)
                nc.vector.tensor_sub(oT[h][:, qsl], d1, d2)

        # ---- output projection: y[st] = sum_h oT_h[:, st]^T @ woT_h
        for st in range(NST):
            ysb = yp.tile([128, E], BF16, tag="ysb")
            ssl = slice(st * 128, (st + 1) * 128)
            for eb in range(4):
                ypp = ps.tile([128, 512], F32, tag="sca" if eb % 2 == 0 else "scb")
                for hh in range(HPC):
                    nc.tensor.matmul(ypp, oT[hh][:, ssl],
                                     woT[hh][:, eb * 512:(eb + 1) * 512],
                                     start=(hh == 0), stop=(hh == HPC - 1))
                nc.vector.tensor_copy(ysb[:, eb * 512:(eb + 1) * 512], ypp)
            nc.sync.dma_start(out=y_d[ssl, :], in_=ysb)

        aactx.close()

    nc.compile()
    return nc


def _lay_lhsT(w):
    # [E_contract, M] f32 -> [128 p, NE c, M] bf16 with e = c*128 + p
    e, m = w.shape
    return np.ascontiguousarray(
        w.reshape(NE, 128, m).transpose(1, 0, 2).astype(ml_dtypes.bfloat16))


def kernel(**inputs):
    x = np.asarray(inputs["x"], dtype=np.float32)
    wq = np.asarray(inputs["wq"], dtype=np.float32)
    wk = np.asarray(inputs["wk"], dtype=np.float32)
    wv = np.asarray(inputs["wv"], dtype=np.float32)
    wq2 = np.asarray(inputs["wq2"], dtype=np.float32)
    wk2 = np.asarray(inputs["wk2"], dtype=np.float32)
    wo = np.asarray(inputs["wo"], dtype=np.float32)
    lq1 = np.asarray(inputs["lambda_q1"], dtype=np.float64)
    lk1 = np.asarray(inputs["lambda_k1"], dtype=np.float64)
    lq2 = np.asarray(inputs["lambda_q2"], dtype=np.float64)
    lk2 = np.asarray(inputs["lambda_k2"], dtype=np.float64)

    lam = float(np.exp(np.sum(lq1 * lk1)) - np.exp(np.sum(lq2 * lk2)) + LAMBDA_INIT)

    if "nc" not in _cached:
        _cached["nc"] = _build()
    nc = _cached["nc"]

    # 128x128 lower-triangular mask for the diagonal blocks
    j = np.arange(128, dtype=np.int32)[None, :]
    i = np.arange(128, dtype=np.int32)[:, None]
    mask1 = np.ascontiguousarray((j >= i).astype(ml_dtypes.bfloat16))
    lam_arr = np.full((1, 1), lam, dtype=np.float32)
    ones = np.ones((128, 1), dtype=ml_dtypes.bfloat16)

    # per-batch x layout: [128 p, NE c, S] with e = c*128 + p
    x_lays = [_lay_lhsT(x[b].T) for b in range(B)]

    in_maps = []
    for c in range(N_CORES):
        b = c // 4
        g = c % 4
        hs = g * HPC * D
        # wqk[p, h] = lhsT chunk layout of W[o_slice, :]^T ([E, 128] -> [128, NE, 128])
        wqk = np.stack([
            np.stack([_lay_lhsT(w[hs + h * D:hs + (h + 1) * D, :].T)
                      for h in range(HPC)])
            for w in (wq, wq2, wk, wk2)])
        wv_lay = _lay_lhsT(wv[hs:hs + HPC * D, :].T)          # [128, NE, 512]
        wo_lay = np.stack([
            np.ascontiguousarray(
                wo[:, hs + h * D:hs + (h + 1) * D].T.astype(ml_dtypes.bfloat16))
            for h in range(HPC)])                              # [4, 128, E]
        in_maps.append({
            "x_lay": x_lays[b],
            "wqk": wqk,
            "wv_lay": wv_lay,
            "wo_lay": wo_lay,
            "mask1": mask1,
            "ones": ones,
            "lam": lam_arr,
        })

    res = bass_utils.run_bass_kernel_spmd(
        nc, in_maps, core_ids=list(range(N_CORES)), trace=TRACE)
    _cached["last_result"] = res

    y = np.zeros((B, S, E), dtype=np.float32)
    for c in range(N_CORES):
        y[c // 4] += np.asarray(res.results[c]["y"], dtype=np.float32)
    return y
